# revision 9
# baseline (speedup 1.0000x reference)
"""Trainium2 Bass kernel for the DCN cross layer.

Computes out = x0 * (x_cross @ w)[:, None] + b + x_cross for
x0, x_cross: [16384, 4096] f32, w, b: [4096] f32.

Sharding: pure data parallel — batch split across 8 NeuronCores,
w and b replicated. Each core processes a [2048, 4096] shard.

The op is memory-bound (3 HBM streams, no reuse) and the f32 version
sits at the 358 GB/s/core DMA roofline, so all I/O is done in bf16:
the host casts inputs once, the device computes in bf16 with an f32
dot-product accumulator, and the host upcasts the output. Error is
~0.1% in norm, well under the 2e-2 gate.
"""

import sys

import numpy as np

sys.path.insert(0, "/opt/trn_rl_repo")

import ml_dtypes

BF16 = ml_dtypes.bfloat16

N_CORES = 8
BATCH = 16384
D = 4096
ROWS_PER_CORE = BATCH // N_CORES  # 2048
P = 128
RPP = 1  # rows per partition per tile -> DMA transfer size = RPP * 1 MB
BUFS = 4

_NC = None


def _build(rpp=None, bufs=None, tmp_bufs=3, s_bufs=4, xcb_eng="gpsimd"):
    """Build + schedule the single-core SPMD program (same on all cores).

    out = x0*(xc.w) + xc + b is refactored as
      xcb = xc + b                       (gpsimd tensor_tensor)
      s'  = rowsum(xcb * w) = s + b.w    (DVE stt+accum, 1x rate)
      s   = s' - beta, beta = b.w        (tiny [P,1] DVE op; beta computed
                                          once on-device, all partitions)
      t   = x0 * s                       (ACT activation, per-part scale)
      out = t + xcb                      (DVE tensor_tensor, 2x rate)
    so the DVE (the old 179us bottleneck) does ~6.4us/tile and the kernel
    is DMA-bound (~143us of bf16 HBM traffic per core).
    """
    from contextlib import ExitStack

    import concourse.tile as tile
    from concourse import bacc, mybir

    rpp = RPP if rpp is None else rpp
    bufs = BUFS if bufs is None else bufs

    bf16 = mybir.dt.bfloat16
    f32 = mybir.dt.float32
    mult = mybir.AluOpType.mult
    add = mybir.AluOpType.add

    nc = bacc.Bacc(
        "TRN2", target_bir_lowering=False, debug=False, num_devices=N_CORES
    )
    x0_d = nc.dram_tensor("x0", [ROWS_PER_CORE, D], bf16, kind="ExternalInput").ap()
    xc_d = nc.dram_tensor(
        "x_cross", [ROWS_PER_CORE, D], bf16, kind="ExternalInput"
    ).ap()
    w_d = nc.dram_tensor("w", [D], bf16, kind="ExternalInput").ap()
    b_d = nc.dram_tensor("b", [D], bf16, kind="ExternalInput").ap()
    out_d = nc.dram_tensor(
        "out", [ROWS_PER_CORE, D], bf16, kind="ExternalOutput"
    ).ap()

    rows_per_tile = P * rpp
    n_tiles = ROWS_PER_CORE // rows_per_tile
    with tile.TileContext(nc) as tc, ExitStack() as ctx:
        consts = ctx.enter_context(tc.tile_pool(name="consts", bufs=1))
        xc_pool = ctx.enter_context(tc.tile_pool(name="xc", bufs=bufs))
        x0_pool = ctx.enter_context(tc.tile_pool(name="x0", bufs=bufs))
        xcb_pool = ctx.enter_context(tc.tile_pool(name="xcb", bufs=tmp_bufs))
        junk_pool = ctx.enter_context(tc.tile_pool(name="junk", bufs=2))
        t_pool = ctx.enter_context(tc.tile_pool(name="t", bufs=tmp_bufs))
        s_pool = ctx.enter_context(tc.tile_pool(name="s", bufs=s_bufs))

        # w and b replicated across all 128 partitions (one-time). The
        # stride-0 DMA broadcast re-reads the same 8 KB per partition but
        # overlaps with the load stream and beat gpsimd.partition_broadcast
        # by ~8 us end-to-end (measured on the f32 version).
        w_t = consts.tile([P, D], bf16)
        b_t = consts.tile([P, D], bf16)
        # issue on the ACT ring (stores come much later there) so the SP
        # ring starts streaming x0/x_cross immediately
        nc.scalar.dma_start(out=w_t[:], in_=w_d.partition_broadcast(P))
        nc.scalar.dma_start(out=b_t[:], in_=b_d.partition_broadcast(P))

        # beta = b.w on every partition (one-time); junk output discarded
        beta_t = consts.tile([P, 1], f32)
        junk0_t = junk_pool.tile([P, D], bf16)
        nc.vector.scalar_tensor_tensor(
            out=junk0_t[:],
            in0=b_t[:],
            scalar=1.0,
            in1=w_t[:],
            op0=mult,
            op1=mult,
            accum_out=beta_t[:],
        )

        xcb_op = nc.gpsimd if xcb_eng == "gpsimd" else nc.vector

        for i in range(n_tiles):
            r0 = i * rows_per_tile
            # [rows_per_tile, D] DRAM block == [P, RPP*D] SBUF tile
            # (partition p holds rows r0 + RPP*p .. r0 + RPP*p + RPP-1)
            xc_t = xc_pool.tile([P, rpp * D], bf16)
            nc.sync.dma_start(
                out=xc_t[:],
                in_=xc_d[r0 : r0 + rows_per_tile, :].rearrange(
                    "(p r) d -> p (r d)", p=P
                ),
            )
            x0_t = x0_pool.tile([P, rpp * D], bf16)
            nc.sync.dma_start(
                out=x0_t[:],
                in_=x0_d[r0 : r0 + rows_per_tile, :].rearrange(
                    "(p r) d -> p (r d)", p=P
                ),
            )

            junk_t = junk_pool.tile([P, D], bf16)
            s_raw = s_pool.tile([P, rpp], f32)
            s_adj = s_pool.tile([P, rpp], f32)
            for j in range(rpp):
                ds = slice(j * D, (j + 1) * D)
                # xcb = xc + b
                xcb_t = xcb_pool.tile([P, D], bf16)
                xcb_op.tensor_add(xcb_t[:], xc_t[:, ds], b_t[:])
                # junk = xcb * w (discarded), s' = rowsum(xcb * w).
                # (TensorScalarPtr fails the Pool-engine ISA check, so the
                # reduce has to stay on the DVE even though it runs at 1x.)
                nc.vector.scalar_tensor_tensor(
                    out=junk_t[:],
                    in0=xcb_t[:],
                    scalar=1.0,
                    in1=w_t[:],
                    op0=mult,
                    op1=mult,
                    accum_out=s_raw[:, j : j + 1],
                )
                # s = s' - beta  (tiny [P,1] op)
                nc.vector.tensor_scalar_sub(
                    s_adj[:, j : j + 1], s_raw[:, j : j + 1], beta_t[:]
                )
                # t = x0 * s on the ACT engine (activation Copy with a
                # per-partition scale AP) to keep the DVE under the DMA
                # roofline budget
                t_t = t_pool.tile([P, D], bf16)
                nc.scalar.mul(t_t[:], x0_t[:, ds], s_adj[:, j : j + 1])
                # out = t + xcb  (2x mode); x0 slice is dead, reuse as output
                nc.vector.tensor_add(x0_t[:, ds], t_t[:], xcb_t[:])
            # store from the ACT HWDGE ring so loads (SP ring) and stores
            # use separate descriptor generators
            nc.scalar.dma_start(
                out=out_d[r0 : r0 + rows_per_tile, :].rearrange(
                    "(p r) d -> p (r d)", p=P
                ),
                in_=x0_t[:],
            )

    nc.compile()
    return nc


def _get_nc():
    global _NC
    if _NC is None:
        _NC = _build()
    return _NC


def _run(inputs, trace=False, **spmd_kwargs):
    """Shard, run on 8 cores, gather. Returns (full_output, BassKernelResults)."""
    from concourse.bass_utils import run_bass_kernel_spmd

    nc = _get_nc()

    x0 = np.ascontiguousarray(np.asarray(inputs["x0"]).astype(BF16))
    xc = np.ascontiguousarray(np.asarray(inputs["x_cross"]).astype(BF16))
    w = np.ascontiguousarray(np.asarray(inputs["w"]).astype(BF16))
    b = np.ascontiguousarray(np.asarray(inputs["b"]).astype(BF16))

    in_maps = [
        {
            "x0": x0[i * ROWS_PER_CORE : (i + 1) * ROWS_PER_CORE],
            "x_cross": xc[i * ROWS_PER_CORE : (i + 1) * ROWS_PER_CORE],
            "w": w,
            "b": b,
        }
        for i in range(N_CORES)
    ]

    res = run_bass_kernel_spmd(
        nc, in_maps, core_ids=list(range(N_CORES)), trace=trace, **spmd_kwargs
    )
    out = np.concatenate(
        [res.results[i]["out"] for i in range(N_CORES)], axis=0
    ).astype(np.float32)
    return out, res


def kernel(**inputs: np.ndarray) -> np.ndarray:
    out, _ = _run(inputs)
    return out


# revision 14
# speedup vs baseline: 1.3200x; 1.3200x over previous
"""Trainium2 Bass kernel for the DCN cross layer.

Computes out = x0 * (x_cross @ w)[:, None] + b + x_cross for
x0, x_cross: [16384, 4096] f32, w, b: [4096] f32.

Sharding: pure data parallel — batch split across 8 NeuronCores,
w and b replicated. Each core processes a [2048, 4096] shard.

The op is memory-bound (3 HBM streams, no reuse) and the f32 version
sits at the 358 GB/s/core DMA roofline, so all I/O is done in bf16:
the host casts inputs once, the device computes in bf16 with an f32
dot-product accumulator, and the host upcasts the output. Error is
~0.1% in norm, well under the 2e-2 gate.
"""

import sys

import numpy as np

sys.path.insert(0, "/opt/trn_rl_repo")

import ml_dtypes

BF16 = ml_dtypes.bfloat16

N_CORES = 8
BATCH = 16384
D = 4096
ROWS_PER_CORE = BATCH // N_CORES  # 2048
P = 128
RPP = 1  # rows per partition per tile -> DMA transfer size = RPP * 1 MB
BUFS = 4

_NC = None


def _build(rpp=None, bufs=None, tmp_bufs=3, s_bufs=4):
    """Build + schedule the single-core SPMD program (same on all cores).

    The DVE runs scalar_tensor_tensor at 1x but tensor_tensor at 2x and
    tensor_scalar at 4x (bf16, packed), and the Pool engine is useless here
    (TensorScalarPtr fails its ISA check; its tensor_tensor measured ~2x
    slower than even the 0.42-efficiency model). So the op is decomposed as
      junk = xc * w                (DVE tensor_tensor, 2x, 2.1us)
      s    = rowsum(junk)          (DVE tensor_scalar *1 + accum_out, 1.1us)
      t    = x0 * s                (ACT activation, per-partition scale AP)
      u    = t + xc                (DVE tensor_tensor, 2x)
      out  = u + b                 (DVE tensor_tensor, 2x)
    which puts the DVE at ~7.5us/tile (120us) and ACT at ~4.7us/tile,
    both under the ~143us of bf16 DMA per core: the kernel is DMA-bound.
    """
    from contextlib import ExitStack

    import concourse.tile as tile
    from concourse import bacc, mybir

    rpp = RPP if rpp is None else rpp
    bufs = BUFS if bufs is None else bufs

    bf16 = mybir.dt.bfloat16
    f32 = mybir.dt.float32
    mult = mybir.AluOpType.mult
    add = mybir.AluOpType.add

    nc = bacc.Bacc(
        "TRN2", target_bir_lowering=False, debug=False, num_devices=N_CORES
    )
    x0_d = nc.dram_tensor("x0", [ROWS_PER_CORE, D], bf16, kind="ExternalInput").ap()
    xc_d = nc.dram_tensor(
        "x_cross", [ROWS_PER_CORE, D], bf16, kind="ExternalInput"
    ).ap()
    w_d = nc.dram_tensor("w", [D], bf16, kind="ExternalInput").ap()
    b_d = nc.dram_tensor("b", [D], bf16, kind="ExternalInput").ap()
    out_d = nc.dram_tensor(
        "out", [ROWS_PER_CORE, D], bf16, kind="ExternalOutput"
    ).ap()

    rows_per_tile = P * rpp
    n_tiles = ROWS_PER_CORE // rows_per_tile
    with tile.TileContext(nc) as tc, ExitStack() as ctx:
        consts = ctx.enter_context(tc.tile_pool(name="consts", bufs=1))
        xc_pool = ctx.enter_context(tc.tile_pool(name="xc", bufs=bufs))
        x0_pool = ctx.enter_context(tc.tile_pool(name="x0", bufs=bufs))
        junk_pool = ctx.enter_context(tc.tile_pool(name="junk", bufs=2))
        t_pool = ctx.enter_context(tc.tile_pool(name="t", bufs=tmp_bufs))
        u_pool = ctx.enter_context(tc.tile_pool(name="u", bufs=tmp_bufs))
        s_pool = ctx.enter_context(tc.tile_pool(name="s", bufs=s_bufs))

        # w and b replicated across all 128 partitions (one-time). The
        # stride-0 DMA broadcast re-reads the same 8 KB per partition but
        # overlaps with the load stream and beat gpsimd.partition_broadcast
        # by ~8 us end-to-end (measured on the f32 version).
        w_t = consts.tile([P, D], bf16)
        b_t = consts.tile([P, D], bf16)
        # issue on the ACT ring (stores come much later there) so the SP
        # ring starts streaming x0/x_cross immediately
        nc.scalar.dma_start(out=w_t[:], in_=w_d.partition_broadcast(P))
        nc.scalar.dma_start(out=b_t[:], in_=b_d.partition_broadcast(P))

        for i in range(n_tiles):
            r0 = i * rows_per_tile
            # [rows_per_tile, D] DRAM block == [P, RPP*D] SBUF tile
            # (partition p holds rows r0 + RPP*p .. r0 + RPP*p + RPP-1)
            xc_t = xc_pool.tile([P, rpp * D], bf16)
            nc.sync.dma_start(
                out=xc_t[:],
                in_=xc_d[r0 : r0 + rows_per_tile, :].rearrange(
                    "(p r) d -> p (r d)", p=P
                ),
            )
            x0_t = x0_pool.tile([P, rpp * D], bf16)
            nc.sync.dma_start(
                out=x0_t[:],
                in_=x0_d[r0 : r0 + rows_per_tile, :].rearrange(
                    "(p r) d -> p (r d)", p=P
                ),
            )

            s_t = s_pool.tile([P, rpp], f32)
            for j in range(rpp):
                ds = slice(j * D, (j + 1) * D)
                # junk = xc * w  (2x mode)
                junk_t = junk_pool.tile([P, D], bf16)
                nc.vector.tensor_tensor(
                    junk_t[:], xc_t[:, ds], w_t[:], mult
                )
                # s = rowsum(junk) via tensor_scalar's accum_out (4x mode;
                # scalar_tensor_tensor would fuse both but runs at 1x)
                junk2_t = junk_pool.tile([P, D], bf16)
                nc.vector.tensor_scalar(
                    out=junk2_t[:],
                    in0=junk_t[:],
                    scalar1=1.0,
                    scalar2=0.0,
                    op0=mult,
                    op1=add,
                    accum_out=s_t[:, j : j + 1],
                )
                # t = x0 * s on the ACT engine (activation Copy with a
                # per-partition scale AP) to keep the DVE under the DMA
                # roofline budget
                t_t = t_pool.tile([P, D], bf16)
                nc.scalar.mul(t_t[:], x0_t[:, ds], s_t[:, j : j + 1])
                # u = t + xc  (2x mode)
                u_t = u_pool.tile([P, D], bf16)
                nc.vector.tensor_add(u_t[:], t_t[:], xc_t[:, ds])
                # out = u + b  (2x mode); x0 slice is dead, reuse as output
                nc.vector.tensor_add(x0_t[:, ds], u_t[:], b_t[:])
            # store from the ACT HWDGE ring so loads (SP ring) and stores
            # use separate descriptor generators
            nc.scalar.dma_start(
                out=out_d[r0 : r0 + rows_per_tile, :].rearrange(
                    "(p r) d -> p (r d)", p=P
                ),
                in_=x0_t[:],
            )

    nc.compile()
    return nc


def _get_nc():
    global _NC
    if _NC is None:
        _NC = _build()
    return _NC


def _run(inputs, trace=False, **spmd_kwargs):
    """Shard, run on 8 cores, gather. Returns (full_output, BassKernelResults)."""
    from concourse.bass_utils import run_bass_kernel_spmd

    nc = _get_nc()

    x0 = np.ascontiguousarray(np.asarray(inputs["x0"]).astype(BF16))
    xc = np.ascontiguousarray(np.asarray(inputs["x_cross"]).astype(BF16))
    w = np.ascontiguousarray(np.asarray(inputs["w"]).astype(BF16))
    b = np.ascontiguousarray(np.asarray(inputs["b"]).astype(BF16))

    in_maps = [
        {
            "x0": x0[i * ROWS_PER_CORE : (i + 1) * ROWS_PER_CORE],
            "x_cross": xc[i * ROWS_PER_CORE : (i + 1) * ROWS_PER_CORE],
            "w": w,
            "b": b,
        }
        for i in range(N_CORES)
    ]

    res = run_bass_kernel_spmd(
        nc, in_maps, core_ids=list(range(N_CORES)), trace=trace, **spmd_kwargs
    )
    out = np.concatenate(
        [res.results[i]["out"] for i in range(N_CORES)], axis=0
    ).astype(np.float32)
    return out, res


def kernel(**inputs: np.ndarray) -> np.ndarray:
    out, _ = _run(inputs)
    return out


# revision 16
# speedup vs baseline: 1.6227x; 1.2294x over previous
"""Trainium2 Bass kernel for the DCN cross layer.

Computes out = x0 * (x_cross @ w)[:, None] + b + x_cross for
x0, x_cross: [16384, 4096] f32, w, b: [4096] f32.

Sharding: pure data parallel — batch split across 8 NeuronCores,
w and b replicated. Each core processes a [2048, 4096] shard.

The op is memory-bound (3 HBM streams, no reuse) and the f32 version
sits at the 358 GB/s/core DMA roofline, so all I/O is done in bf16:
the host casts inputs once, the device computes in bf16 with an f32
dot-product accumulator, and the host upcasts the output. Error is
~0.1% in norm, well under the 2e-2 gate.
"""

import sys

import numpy as np

sys.path.insert(0, "/opt/trn_rl_repo")

import ml_dtypes

BF16 = ml_dtypes.bfloat16

N_CORES = 8
BATCH = 16384
D = 4096
ROWS_PER_CORE = BATCH // N_CORES  # 2048
P = 128
RPP = 1  # rows per partition per tile -> DMA transfer size = RPP * 1 MB
BUFS = 4

_NC = None


def _build(rpp=None, bufs=None, tmp_bufs=3, s_bufs=4, reduce_mode="stt"):
    """Build + schedule the single-core SPMD program (same on all cores).

    Engine facts measured on HW: DVE tensor_tensor runs at 2x (2.29us per
    [128,4096] bf16 tile), but every DVE op with a free-dim reduction
    (scalar_tensor_tensor, TENSOR_SCALAR_CACHE_REDUCE) runs at 1x
    (~4.4us). ACT activation is 3.8us/tile and also has an accum_out
    rowsum. The Pool engine is useless here (TensorScalarPtr fails its ISA
    check; its tensor_tensor measured ~2x slower than the model).

    reduce_mode='stt':  DVE stt junk=xc*w + accum s (4.4us)
    reduce_mode='act':  DVE tt junk=xc*w (2.3us), ACT Copy(junk)+accum s
                        (3.8us) - moves the reduce cost to the ACT engine

    Then t = x0*s on ACT (per-partition scale AP), u = t+xc and out = u+b
    on DVE at 2x. The final adds for tile i are emitted one iteration
    late (software pipelining): each engine's queue executes in program
    order, so without this the DVE sits in u_i waiting on ACT's t_i while
    the ready stt_{i+1} is stuck behind it in the queue.
    """
    from contextlib import ExitStack

    import concourse.tile as tile
    from concourse import bacc, mybir

    rpp = RPP if rpp is None else rpp
    bufs = BUFS if bufs is None else bufs

    bf16 = mybir.dt.bfloat16
    f32 = mybir.dt.float32
    mult = mybir.AluOpType.mult
    add = mybir.AluOpType.add

    nc = bacc.Bacc(
        "TRN2", target_bir_lowering=False, debug=False, num_devices=N_CORES
    )
    x0_d = nc.dram_tensor("x0", [ROWS_PER_CORE, D], bf16, kind="ExternalInput").ap()
    xc_d = nc.dram_tensor(
        "x_cross", [ROWS_PER_CORE, D], bf16, kind="ExternalInput"
    ).ap()
    w_d = nc.dram_tensor("w", [D], bf16, kind="ExternalInput").ap()
    b_d = nc.dram_tensor("b", [D], bf16, kind="ExternalInput").ap()
    out_d = nc.dram_tensor(
        "out", [ROWS_PER_CORE, D], bf16, kind="ExternalOutput"
    ).ap()

    rows_per_tile = P * rpp
    n_tiles = ROWS_PER_CORE // rows_per_tile
    with tile.TileContext(nc) as tc, ExitStack() as ctx:
        consts = ctx.enter_context(tc.tile_pool(name="consts", bufs=1))
        xc_pool = ctx.enter_context(tc.tile_pool(name="xc", bufs=bufs))
        x0_pool = ctx.enter_context(tc.tile_pool(name="x0", bufs=bufs))
        junk_pool = ctx.enter_context(tc.tile_pool(name="junk", bufs=2))
        t_pool = ctx.enter_context(tc.tile_pool(name="t", bufs=tmp_bufs))
        u_pool = ctx.enter_context(tc.tile_pool(name="u", bufs=tmp_bufs))
        s_pool = ctx.enter_context(tc.tile_pool(name="s", bufs=s_bufs))

        # w and b replicated across all 128 partitions (one-time). The
        # stride-0 DMA broadcast re-reads the same 8 KB per partition but
        # overlaps with the load stream and beat gpsimd.partition_broadcast
        # by ~8 us end-to-end (measured on the f32 version).
        w_t = consts.tile([P, D], bf16)
        b_t = consts.tile([P, D], bf16)
        # issue on the ACT ring (stores come much later there) so the SP
        # ring starts streaming x0/x_cross immediately
        nc.scalar.dma_start(out=w_t[:], in_=w_d.partition_broadcast(P))
        nc.scalar.dma_start(out=b_t[:], in_=b_d.partition_broadcast(P))

        assert rpp == 1, "software-pipelined loop assumes rpp == 1"

        def finish(prev):
            """Emit tile i's ACT-dependent tail (u, out, store)."""
            xc_p, x0_p, t_p, r0_p = prev
            # u = t + xc  (2x mode)
            u_t = u_pool.tile([P, D], bf16)
            nc.vector.tensor_add(u_t[:], t_p[:], xc_p[:])
            # out = u + b  (2x mode); x0 is dead, reuse it as the output
            nc.vector.tensor_add(x0_p[:], u_t[:], b_t[:])
            # store from the ACT HWDGE ring so loads (SP ring) and stores
            # use separate descriptor generators
            nc.scalar.dma_start(
                out=out_d[r0_p : r0_p + rows_per_tile, :].rearrange(
                    "(p r) d -> p (r d)", p=P
                ),
                in_=x0_p[:],
            )

        prev = None
        for i in range(n_tiles):
            r0 = i * rows_per_tile
            # [rows_per_tile, D] DRAM block == [P, D] SBUF tile
            xc_t = xc_pool.tile([P, D], bf16)
            nc.sync.dma_start(
                out=xc_t[:],
                in_=xc_d[r0 : r0 + rows_per_tile, :].rearrange(
                    "(p r) d -> p (r d)", p=P
                ),
            )
            x0_t = x0_pool.tile([P, D], bf16)
            nc.sync.dma_start(
                out=x0_t[:],
                in_=x0_d[r0 : r0 + rows_per_tile, :].rearrange(
                    "(p r) d -> p (r d)", p=P
                ),
            )

            s_t = s_pool.tile([P, 1], f32)
            if reduce_mode == "stt":
                # junk = xc * w (discarded), s = rowsum(xc * w), 1x rate
                junk_t = junk_pool.tile([P, D], bf16)
                nc.vector.scalar_tensor_tensor(
                    out=junk_t[:],
                    in0=xc_t[:],
                    scalar=1.0,
                    in1=w_t[:],
                    op0=mult,
                    op1=mult,
                    accum_out=s_t[:],
                )
            else:
                # junk = xc * w on DVE (2x), rowsum on ACT via Copy+accum
                junk_t = junk_pool.tile([P, D], bf16)
                nc.vector.tensor_tensor(junk_t[:], xc_t[:], w_t[:], mult)
                junk2_t = junk_pool.tile([P, D], bf16)
                nc.scalar.activation(
                    out=junk2_t[:],
                    in_=junk_t[:],
                    func=mybir.ActivationFunctionType.Copy,
                    accum_out=s_t[:],
                )
            # t = x0 * s on the ACT engine (activation Copy with a
            # per-partition scale AP)
            t_t = t_pool.tile([P, D], bf16)
            nc.scalar.mul(t_t[:], x0_t[:], s_t[:])

            if prev is not None:
                finish(prev)
            prev = (xc_t, x0_t, t_t, r0)
        finish(prev)

    nc.compile()
    return nc


def _get_nc():
    global _NC
    if _NC is None:
        _NC = _build()
    return _NC


def _run(inputs, trace=False, **spmd_kwargs):
    """Shard, run on 8 cores, gather. Returns (full_output, BassKernelResults)."""
    from concourse.bass_utils import run_bass_kernel_spmd

    nc = _get_nc()

    x0 = np.ascontiguousarray(np.asarray(inputs["x0"]).astype(BF16))
    xc = np.ascontiguousarray(np.asarray(inputs["x_cross"]).astype(BF16))
    w = np.ascontiguousarray(np.asarray(inputs["w"]).astype(BF16))
    b = np.ascontiguousarray(np.asarray(inputs["b"]).astype(BF16))

    in_maps = [
        {
            "x0": x0[i * ROWS_PER_CORE : (i + 1) * ROWS_PER_CORE],
            "x_cross": xc[i * ROWS_PER_CORE : (i + 1) * ROWS_PER_CORE],
            "w": w,
            "b": b,
        }
        for i in range(N_CORES)
    ]

    res = run_bass_kernel_spmd(
        nc, in_maps, core_ids=list(range(N_CORES)), trace=trace, **spmd_kwargs
    )
    out = np.concatenate(
        [res.results[i]["out"] for i in range(N_CORES)], axis=0
    ).astype(np.float32)
    return out, res


def kernel(**inputs: np.ndarray) -> np.ndarray:
    out, _ = _run(inputs)
    return out
